# revision 1
# baseline (speedup 1.0000x reference)
"""Causal multi-head self-attention (B=2, S=2048, D=768, H=12) on 8 TRN2 NeuronCores.

Sharding: core c = (batch b=c//4, head-group hg=c%4 of 3 heads).
Each core computes Q/K/V for its 3 heads, causal attention, and the partial
output projection sum_h out_h @ Wo[:, h]^T -> (S, D). Host sums the 4
head-group partials per batch (the unshard step).

On-core dataflow (transposed (feature, seq) layout, f32r matmuls):
  A) QKV^T: psum[m, s] += WcatT[i, m].T @ XT[i, s]   (i outer-of-sc so one
     stationary serves 4 moving matmuls; starts as soon as xt chunk 0 lands)
  B) V natural: PE-transpose V^T tiles -> V' = [V | ones] per k-tile
  C) per head, per q-half qp (2 q-chunks of 512), per k-tile t:
       scoresT[k, q] = KT[:,t].T @ QT   (only causally-valid halves)
       additive -30000 mask on the diagonal half, exp on ACT -> f32r,
       PV: pout[qc] += V'[t].T @ expT   (65 rows: 64 data + denominator)
     then per qc: recip(den) -> broadcast -> numerator * recip -> outcatT
  D) projection: psum[q, j] += outcatT[h, q].T @ WoT[h, j]; copy; DMA out.

PSUM budget in C: score tiles (128,1024)=2 banks x3 bufs + 2 pout banks = 8.
"""

import numpy as np
from contextlib import ExitStack

import concourse.bass as bass
import concourse.tile as tile
from concourse import bacc, mybir
from concourse import bass_utils

F32 = mybir.dt.float32
F32R = mybir.dt.float32r
BF16 = mybir.dt.bfloat16
FP16 = mybir.dt.float16
AF = mybir.ActivationFunctionType

B, S, D, H = 2, 2048, 768, 12
DK = 64
HPC = 3            # heads per core
NCORES = 8
NI = D // 128      # 6 input-feature chunks
NM = 5             # output m-chunks of 128 (640 rows incl. 64 pad)
NT = S // 128      # 16 k-tiles
NQC = S // 512     # 4 q-chunks
MASK_NEG = -30000.0

# per-local-head (base_partition, m_chunk) in the QKVT buffer
QPOS = [(0, 0), (64, 0), (0, 2)]
KPOS = [(0, 1), (64, 1), (0, 3)]
VPOS = [(64, 2), (64, 3), (0, 4)]

_NC_CACHE = {}


def _enable_ldw_opt():
    """Let walrus dedupe back-to-back identical weight loads (verified
    bit-identical output on this kernel; saves ~1/3 of LDWEIGHTS)."""
    if getattr(bass_utils.run_command, "_ldw_patched", False):
        return
    orig = bass_utils.run_command

    def patched(argv, **kw):
        argv = ["--enable-ldw-opt=true" if a == "--enable-ldw-opt=false" else a
                for a in argv]
        return orig(argv, **kw)

    patched._ldw_patched = True
    bass_utils.run_command = patched


def build_nc(dbg=False):
    key = ("nc", dbg)
    if key in _NC_CACHE:
        return _NC_CACHE[key]
    # ldw-opt incompatible with fp16 matmul explicit ldweights
    nc = bacc.Bacc("TRN2", target_bir_lowering=False, debug=False,
                   num_devices=NCORES)

    xt_d = nc.dram_tensor("xt", [NI, 128, S], F32R, kind="ExternalInput").ap()
    wcat_d = nc.dram_tensor("wcat", [NI, 128, NM * 128], F32R, kind="ExternalInput").ap()
    wot_d = nc.dram_tensor("wot", [2, 128, D], FP16, kind="ExternalInput").ap()
    mask_d = nc.dram_tensor("mask", [128, 128], F32, kind="ExternalInput").ap()
    id_d = nc.dram_tensor("ident", [128, 128], F32R, kind="ExternalInput").ap()
    ones_d = nc.dram_tensor("vones", [128, HPC * NT], FP16, kind="ExternalInput").ap()
    out_d = nc.dram_tensor("out", [S, D], F32, kind="ExternalOutput").ap()
    if dbg:
        qkv_dbg = nc.dram_tensor("qkv_dbg", [128, NM, S], F32, kind="ExternalOutput").ap()
        vp_dbg = nc.dram_tensor("vp_dbg", [128, HPC, NT, DK + 1], F32, kind="ExternalOutput").ap()
        oct_dbg = nc.dram_tensor("oct_dbg", [DK, HPC, S], F32, kind="ExternalOutput").ap()

    with tile.TileContext(nc) as tc, ExitStack() as ctx:
        const = ctx.enter_context(tc.tile_pool(name="const", bufs=1))

        # persistent SBUF buffers
        xt = const.tile([128, NI, S], F32R)             # X^T
        wcat = const.tile([128, NI, NM * 128], F32R)    # W^T (QKV packed)
        wot = const.tile([128, 2, D], FP16)             # Wo^T [h0;h1],[h2;pad]
        maskb = const.tile([128, 128], F32)             # diag causal bias tile
        ident = const.tile([128, 128], F32R)
        qkvt = const.tile([128, NM, S], F32R)           # Q^T/K^T/V^T packed
        vp = const.tile([128, HPC, NT, DK + 1], FP16)   # V' = [V | ones]
        oct_ = const.tile([128, 2, S], FP16)            # packed out^T [h0;h1],[h2]
        qk16 = const.tile([128, 4, S], FP16)            # fp16 Q/K for attention

        # priority: what phase A's first accumulation chains touch first
        for i in range(NI):
            nc.sync.dma_start(wcat[:, i, 0:128], wcat_d[i][:, 0:128])
        for sh in range(4):
            for i in range(NI):
                nc.sync.dma_start(xt[:, i, sh * 512:(sh + 1) * 512],
                                  xt_d[i][:, sh * 512:(sh + 1) * 512])
        for i in range(NI):
            nc.sync.dma_start(wcat[:, i, 128:NM * 128], wcat_d[i][:, 128:NM * 128])
        nc.sync.dma_start(ident[:], id_d)
        nc.sync.dma_start(vp[:, :, :, DK:DK + 1],
                          ones_d.rearrange("p (h t) -> p h t", h=HPC))
        nc.sync.dma_start(maskb[:], mask_d)
        nc.sync.dma_start(wot[:], wot_d.rearrange("c p f -> p c f"))

        # ---- Phase A: QKV^T projection; Phase B: V transposes (shared pool)
        with tc.tile_pool(name="ps_ab", bufs=4, space="PSUM") as ps_ab:
            for m in range(NM):
                pqs = [ps_ab.tile([128, 512], F32, tag="proj", name=f"pq{m}_{sc}")
                       for sc in range(NQC)]
                for i in range(NI):
                    for sc in range(NQC):
                        nc.tensor.matmul(
                            pqs[sc][:],
                            wcat[:, i, m * 128:(m + 1) * 128],
                            xt[:, i, sc * 512:(sc + 1) * 512],
                            start=(i == 0), stop=(i == NI - 1),
                        )
                for sc in range(NQC):
                    nc.vector.tensor_copy(
                        qkvt[:, m, sc * 512:(sc + 1) * 512], pqs[sc][:])
                # fp16 shadow of Q/K rows for the attention core
                if m <= 1:
                    for sc in range(NQC):
                        nc.vector.tensor_copy(
                            qk16[:, m, sc * 512:(sc + 1) * 512],
                            pqs[sc][:])
                elif m <= 3:
                    for sc in range(NQC):
                        nc.vector.tensor_copy(
                            qk16[0:DK, m, sc * 512:(sc + 1) * 512],
                            pqs[sc][0:DK, :])

            for h in range(HPC):
                vb, vchunk = VPOS[h]
                for t in range(NT):
                    ptr = ps_ab.tile([128, DK], F32R, tag="tr", bufs=4,
                                     name=f"tr{h}_{t}")
                    nc.tensor.transpose(
                        ptr[:],
                        qkvt[vb:vb + DK, vchunk, t * 128:(t + 1) * 128],
                        ident[vb:vb + DK, vb:vb + DK],
                    )
                    nc.vector.tensor_copy(vp[:, h, t, 0:DK], ptr[:])

        # ---- Phase C: attention per head, q-half outer (pscr triple-buffered)
        with tc.tile_pool(name="ps_s", bufs=3, space="PSUM") as ps_s, \
             tc.tile_pool(name="ps_o", bufs=2, space="PSUM") as ps_o, \
             tc.tile_pool(name="sb_exp", bufs=6) as sb_exp, \
             tc.tile_pool(name="sb_div", bufs=3) as sb_div:
            for h in range(HPC):
                qb, qchunk = QPOS[h]
                kb, kchunk = KPOS[h]
                pouts = {}

                def score_step(qp, t):
                    qcs = (2 * qp, 2 * qp + 1)
                    qc_lo = t // 4
                    off = 128 * (t % 4)   # diag col offset inside qc_lo's half
                    pscr = ps_s.tile([128, 1024], F32, tag="scr",
                                     name=f"sc{h}_{qp}_{t}")
                    for half, qc in enumerate(qcs):
                        if qc < qc_lo:
                            continue
                        cs = off if qc == qc_lo else 0  # skip fully-masked cols
                        nc.tensor.matmul(
                            pscr[:, half * 512 + cs:(half + 1) * 512],
                            qk16[kb:kb + DK, kchunk, t * 128:(t + 1) * 128],
                            qk16[qb:qb + DK, qchunk,
                                 qc * 512 + cs:(qc + 1) * 512],
                            start=True, stop=True,
                        )
                    if qc_lo in qcs:  # mask only the 128-wide diagonal window
                        half = qc_lo - 2 * qp
                        nc.vector.tensor_add(
                            pscr[:, half * 512 + off:half * 512 + off + 128],
                            pscr[:, half * 512 + off:half * 512 + off + 128],
                            maskb[:, 0:128],
                        )
                    lo = (512 if qc_lo == qcs[1] else 0) + \
                         (off if qc_lo in qcs else 0)
                    expt = sb_exp.tile([128, 1024], FP16, tag="exp",
                                       name=f"ex{h}_{qp}_{t}")
                    nc.scalar.activation(expt[:, lo:1024], pscr[:, lo:1024],
                                         AF.Exp)
                    return expt

                def pv_step(qp, t, expt):
                    qcs = (2 * qp, 2 * qp + 1)
                    qc_lo = t // 4
                    off = 128 * (t % 4)
                    for half, qc in enumerate(qcs):
                        if qc < qc_lo:
                            continue
                        cs = off if qc == qc_lo else 0
                        nc.tensor.matmul(
                            pouts[qc][:, cs:512],
                            vp[:, h, t, :],
                            expt[:, half * 512 + cs:(half + 1) * 512],
                            start=(t == 0), stop=(t == 4 * qc + 3),
                        )

                def divide(qc):
                    # evict the finished chain at once so its PSUM bank frees
                    # immediately; the slow recip/divide runs off the copy
                    nout = sb_div.tile([DK + 1, 512], F32, tag="nout",
                                       name=f"no{h}_{qc}")
                    nc.vector.tensor_copy(nout[:], pouts[qc][:])
                    # spread the 512-wide den row over 64 partitions so the
                    # expensive reciprocal runs 64 lanes wide, not 1
                    rsp = sb_div.tile([DK, 8], F32, tag="rsp",
                                      name=f"rsp{h}_{qc}")
                    nc.sync.dma_start(rsp[:], nout[DK:DK + 1, :])
                    rcs = sb_div.tile([DK, 8], F32, tag="rcs",
                                      name=f"rcs{h}_{qc}")
                    nc.vector.reciprocal(rcs[:], rsp[:])
                    rc0 = sb_div.tile([1, 512], F32, tag="rc0",
                                      name=f"rc0{h}_{qc}")
                    nc.sync.dma_start(rc0[:], rcs[:])
                    rb = sb_div.tile([DK, 512], F32, tag="rb",
                                     name=f"rb{h}_{qc}")
                    nc.gpsimd.partition_broadcast(rb[:], rc0[:])
                    if h == 1:
                        # h1 lands at partitions 64-127: shift via SBUF DMA
                        tmp = sb_div.tile([DK, 512], FP16, tag="tmp",
                                          name=f"tmp{h}_{qc}")
                        nc.vector.tensor_mul(tmp[:], nout[0:DK, :], rb[:])
                        nc.sync.dma_start(
                            oct_[DK:128, 0, qc * 512:(qc + 1) * 512], tmp[:])
                    else:
                        nc.vector.tensor_mul(
                            oct_[0:DK, h // 2, qc * 512:(qc + 1) * 512],
                            nout[0:DK, :], rb[:],
                        )

                # pair the k-tiles: two same-geometry score LDW+MMs
                # back-to-back, then two same-geometry PV LDW+MMs — halves
                # PE stationary-geometry switches (measured ~2x matmul cost
                # per switch in isolation)
                for qp in range(2):
                    for qc in (2 * qp, 2 * qp + 1):
                        pouts[qc] = ps_o.tile([DK + 1, 512], F32, tag="pout",
                                              name=f"po{h}_{qc}")
                    for t0 in range(0, 4 * (2 * qp + 1) + 4, 2):
                        e0 = score_step(qp, t0)
                        e1 = score_step(qp, t0 + 1)
                        pv_step(qp, t0, e0)
                        pv_step(qp, t0 + 1, e1)
                        if t0 + 1 == 4 * (2 * qp) + 3:
                            divide(2 * qp)      # low chain done: free its bank
                    divide(2 * qp + 1)

            # ---- output projection, reusing the attention pools' slots
            for qt in range(NT):
                pp = ps_s.tile([128, D], F32, tag="scr", name=f"pp{qt}")
                for c, kk in ((0, 128), (1, DK)):
                    for js, je in ((0, 512), (512, D)):
                        nc.tensor.matmul(
                            pp[:, js:je],
                            oct_[0:kk, c, qt * 128:(qt + 1) * 128],
                            wot[0:kk, c, js:je],
                            start=(c == 0), stop=(c == 1),
                        )
                ot = sb_exp.tile([128, D], F32, tag="exp", name=f"ot{qt}")
                nc.vector.tensor_copy(ot[:], pp[:])
                nc.sync.dma_start(out_d[qt * 128:(qt + 1) * 128, :], ot[:])

        if dbg:
            nc.sync.dma_start(qkv_dbg, qkvt[:].bitcast(F32))
            nc.sync.dma_start(vp_dbg, vp[:].bitcast(F32))
            nc.sync.dma_start(oct_dbg, oct_[:].bitcast(F32))


    nc.compile()
    _NC_CACHE[key] = nc
    return nc


def make_in_maps(X, Wq, Wk, Wv, Wo):
    X = np.ascontiguousarray(np.asarray(X, dtype=np.float32))
    Wq = np.asarray(Wq, dtype=np.float32)
    Wk = np.asarray(Wk, dtype=np.float32)
    Wv = np.asarray(Wv, dtype=np.float32)
    Wo = np.asarray(Wo, dtype=np.float32)

    # causal additive-bias tiles: keep q >= k; rows=k (p), cols=q (f),
    # diagonal offset delta = 128*di: keep iff f >= p + delta
    p = np.arange(128)[:, None]
    f = np.arange(512)[None, :]
    mask = np.where(f[:, :128] >= p, 0.0, MASK_NEG).astype(np.float32)
    ident = np.eye(128, dtype=np.float32)
    vones = np.ones((128, HPC * NT), dtype=np.float16)

    in_maps = []
    for c in range(NCORES):
        b, hg = c // 4, c % 4
        gh = [hg * HPC + l for l in range(HPC)]
        q = [Wq[g * DK:(g + 1) * DK, :] / 8.0 for g in gh]
        k = [Wk[g * DK:(g + 1) * DK, :] for g in gh]
        v = [Wv[g * DK:(g + 1) * DK, :] for g in gh]
        wcat_rows = np.vstack([
            q[0], q[1], k[0], k[1], q[2], v[0], k[2], v[1], v[2],
            np.zeros((DK, D), dtype=np.float32),
        ])                                            # (640, 768)
        wcat = np.ascontiguousarray(wcat_rows.T.reshape(NI, 128, NM * 128))
        w0, w1, w2 = (Wo[:, g * DK:(g + 1) * DK].T for g in gh)
        wot = np.ascontiguousarray(np.stack([
            np.vstack([w0, w1]),
            np.vstack([w2, np.zeros((DK, D), dtype=np.float32)]),
        ]).astype(np.float16))                                     # (2, 128, 768)
        xt = np.ascontiguousarray(X[b].T.reshape(NI, 128, S))
        in_maps.append({
            "xt": xt, "wcat": wcat, "wot": wot,
            "mask": mask, "ident": ident, "vones": vones,
        })
    return in_maps


def _run(in_maps, trace=False, trace_cores=None):
    nc = build_nc()
    return bass_utils.run_bass_kernel_spmd(
        nc, in_maps, core_ids=list(range(NCORES)),
        trace=trace, trace_cores=trace_cores,
    )


def kernel(X, Wq, Wk, Wv, Wo):
    in_maps = make_in_maps(X, Wq, Wk, Wv, Wo)
    res = _run(in_maps, trace=False)
    out = np.zeros((B, S, D), dtype=np.float32)
    for c in range(NCORES):
        out[c // 4] += res.results[c]["out"]
    return out



# revision 3
# speedup vs baseline: 1.1013x; 1.1013x over previous
"""Causal multi-head self-attention (B=2, S=2048, D=768, H=12) on 8 TRN2 NeuronCores.

Sharding: core c = (batch b=c//4, head-group hg=c%4 of 3 heads).
Each core computes Q/K/V for its 3 heads, causal attention, and the partial
output projection sum_h out_h @ Wo[:, h]^T -> (S, D) in fp16. Host sums the
4 head-group partials per batch (the unshard step).

On-core dataflow (transposed (feature, seq) layout, f32r matmuls):
  A) QKV^T, sc-outer: for each 512-col s-chunk, 5 m-chunks of
     psum[m, s] += WcatT[i, m].T @ XT[i, s] over i; evict + fp16 shadows.
     sc-outer lets compute start as soon as the first quarter of X lands
     (DMAs are issued from sync+scalar+gpsimd to parallelize descriptor
     generation) and keeps an 8-bank PSUM rotation so the PE never stalls
     on evictions.
  B) V' = [V | ones] built per s-chunk right after its V rows exist
     (PE transpose + DVE copy), keeping the PE stream dense.
  C) attention, software-pipelined: a flat stream of score steps
     (scoresT[k, q] = KT.T @ QT per 1024-wide q-half, additive -30000 mask
     on the diagonal window, exp on ACT -> fp16) runs R=3 steps AHEAD of
     the PV stream (pout[qc] += V'[t].T @ expT, 65 rows: 64 data +
     denominator). The PE therefore always has score matmuls queued while
     ACT exps, instead of round-tripping score->exp->PV per tile.
     Per-(h,qc) epilogue: recip(den) -> broadcast -> numerator * recip.
  D) projection: psum[q, j] += outcatT[h, q].T @ WoT[h, j]; evict fp16;
     DMA out (fp16 partials; host accumulates in f32).

PSUM budget in C: score tiles (128,1024)=2 banks x3 bufs + 2 pout banks = 8.
"""

import numpy as np
from contextlib import ExitStack

import concourse.bass as bass
import concourse.tile as tile
from concourse import bacc, mybir
from concourse import bass_utils

F32 = mybir.dt.float32
F32R = mybir.dt.float32r
BF16 = mybir.dt.bfloat16
FP16 = mybir.dt.float16
AF = mybir.ActivationFunctionType

B, S, D, H = 2, 2048, 768, 12
DK = 64
HPC = 3            # heads per core
NCORES = 8
NI = D // 128      # 6 input-feature chunks
NM = 5             # output m-chunks of 128 (640 rows incl. 64 pad)
NT = S // 128      # 16 k-tiles
NQC = S // 512     # 4 q-chunks
MASK_NEG = -30000.0

# per-local-head (base_partition, m_chunk) in the QKVT buffer
QPOS = [(0, 0), (64, 0), (0, 2)]
KPOS = [(0, 1), (64, 1), (0, 3)]
VPOS = [(64, 2), (64, 3), (0, 4)]

_NC_CACHE = {}


def _enable_ldw_opt():
    """Let walrus dedupe back-to-back identical weight loads (verified
    bit-identical output on this kernel; saves ~1/3 of LDWEIGHTS)."""
    if getattr(bass_utils.run_command, "_ldw_patched", False):
        return
    orig = bass_utils.run_command

    def patched(argv, **kw):
        argv = ["--enable-ldw-opt=true" if a == "--enable-ldw-opt=false" else a
                for a in argv]
        return orig(argv, **kw)

    patched._ldw_patched = True
    bass_utils.run_command = patched


def build_nc(dbg=False):
    key = ("nc", dbg)
    if key in _NC_CACHE:
        return _NC_CACHE[key]
    nc = bacc.Bacc("TRN2", target_bir_lowering=False, debug=False,
                   num_devices=NCORES)

    xt_d = nc.dram_tensor("xt", [NI, 128, S], F32R, kind="ExternalInput").ap()
    wcat_d = nc.dram_tensor("wcat", [NI, 128, NM * 128], F32R, kind="ExternalInput").ap()
    wot_d = nc.dram_tensor("wot", [2, 128, D], FP16, kind="ExternalInput").ap()
    mask_d = nc.dram_tensor("mask", [128, 128], F32, kind="ExternalInput").ap()
    id_d = nc.dram_tensor("ident", [128, 128], F32R, kind="ExternalInput").ap()
    ones_d = nc.dram_tensor("vones", [128, HPC * NT], FP16, kind="ExternalInput").ap()
    out_d = nc.dram_tensor("out", [S, D], FP16, kind="ExternalOutput").ap()
    if dbg:
        qkv_dbg = nc.dram_tensor("qkv_dbg", [128, NM, S], F32, kind="ExternalOutput").ap()
        vp_dbg = nc.dram_tensor("vp_dbg", [128, HPC, NT, DK + 1], F32, kind="ExternalOutput").ap()
        oct_dbg = nc.dram_tensor("oct_dbg", [DK, HPC, S], F32, kind="ExternalOutput").ap()

    with tile.TileContext(nc) as tc, ExitStack() as ctx:
        const = ctx.enter_context(tc.tile_pool(name="const", bufs=1))

        # persistent SBUF buffers
        xt = const.tile([128, NI, S], F32R)             # X^T
        wcat = const.tile([128, NI, NM * 128], F32R)    # W^T (QKV packed)
        wot = const.tile([128, 2, D], FP16)             # Wo^T [h0;h1],[h2;pad]
        maskb = const.tile([128, 128], F32)             # diag causal bias tile
        ident = const.tile([128, 128], F32R)
        qkvt = const.tile([128, NM, S], F32R)           # Q^T/K^T/V^T packed
        vp = const.tile([128, HPC, NT, DK + 1], FP16)   # V' = [V | ones]
        oct_ = const.tile([128, 2, S], FP16)            # packed out^T [h0;h1],[h2]
        qk16 = const.tile([128, 4, S], FP16)            # fp16 Q/K for attention

        # DMA issue order = arrival priority. Spread descriptor generation
        # over three otherwise-idle engine queues; phase A's critical path
        # (wcat m-chunk columns + the first s-chunk of X) goes first.
        ENGS = (nc.sync, nc.scalar, nc.gpsimd)
        for i in range(NI):
            ENGS[i % 3].dma_start(wcat[:, i, 0:128], wcat_d[i][:, 0:128])
        for i in range(NI):
            ENGS[i % 3].dma_start(xt[:, i, 0:512], xt_d[i][:, 0:512])
        for i in range(NI):
            ENGS[i % 3].dma_start(wcat[:, i, 128:NM * 128],
                                  wcat_d[i][:, 128:NM * 128])
        nc.sync.dma_start(ident[:], id_d)
        nc.scalar.dma_start(vp[:, :, :, DK:DK + 1],
                            ones_d.rearrange("p (h t) -> p h t", h=HPC))
        nc.gpsimd.dma_start(maskb[:], mask_d)
        nc.sync.dma_start(wot[:], wot_d.rearrange("c p f -> p c f"))
        for sc in range(1, NQC):
            for i in range(NI):
                ENGS[(sc * NI + i) % 3].dma_start(
                    xt[:, i, sc * 512:(sc + 1) * 512],
                    xt_d[i][:, sc * 512:(sc + 1) * 512])

        # ---- Phase A+B fused, sc-outer: QKV^T projection + V' transposes
        with tc.tile_pool(name="ps_ab", bufs=8, space="PSUM") as ps_ab:
            for sc in range(NQC):
                s0 = sc * 512
                for m in range(NM):
                    pq = ps_ab.tile([128, 512], F32, tag="ab",
                                    name=f"pq{sc}_{m}")
                    for i in range(NI):
                        nc.tensor.matmul(
                            pq[:],
                            wcat[:, i, m * 128:(m + 1) * 128],
                            xt[:, i, s0:s0 + 512],
                            start=(i == 0), stop=(i == NI - 1),
                        )
                    nc.vector.tensor_copy(qkvt[:, m, s0:s0 + 512], pq[:])
                    # fp16 shadow of Q/K rows for the attention core
                    if m <= 1:
                        nc.vector.tensor_copy(qk16[:, m, s0:s0 + 512], pq[:])
                    elif m <= 3:
                        nc.vector.tensor_copy(qk16[0:DK, m, s0:s0 + 512],
                                              pq[0:DK, :])
                for h in range(HPC):
                    vb, vchunk = VPOS[h]
                    for t in range(4 * sc, 4 * sc + 4):
                        ptr = ps_ab.tile([128, DK], F32R, tag="ab",
                                         name=f"tr{h}_{t}")
                        nc.tensor.transpose(
                            ptr[:],
                            qkvt[vb:vb + DK, vchunk, t * 128:(t + 1) * 128],
                            ident[vb:vb + DK, vb:vb + DK],
                        )
                        nc.vector.tensor_copy(vp[:, h, t, 0:DK], ptr[:])

        # ---- Phase C: pipelined attention; Phase D: output projection
        with tc.tile_pool(name="ps_s", bufs=3, space="PSUM") as ps_s, \
             tc.tile_pool(name="ps_o", bufs=2, space="PSUM") as ps_o, \
             tc.tile_pool(name="sb_exp", bufs=6) as sb_exp, \
             tc.tile_pool(name="sb_div", bufs=3) as sb_div:

            pouts = {}

            def score_step(h, qp, t):
                qb, qchunk = QPOS[h]
                kb, kchunk = KPOS[h]
                qcs = (2 * qp, 2 * qp + 1)
                qc_lo = t // 4
                off = 128 * (t % 4)   # diag col offset inside qc_lo's half
                pscr = ps_s.tile([128, 1024], F32, tag="scr",
                                 name=f"sc{h}_{qp}_{t}")
                for half, qc in enumerate(qcs):
                    if qc < qc_lo:
                        continue
                    cs = off if qc == qc_lo else 0  # skip fully-masked cols
                    nc.tensor.matmul(
                        pscr[:, half * 512 + cs:(half + 1) * 512],
                        qk16[kb:kb + DK, kchunk, t * 128:(t + 1) * 128],
                        qk16[qb:qb + DK, qchunk,
                             qc * 512 + cs:(qc + 1) * 512],
                        start=True, stop=True,
                    )
                if qc_lo in qcs:  # mask only the 128-wide diagonal window
                    half = qc_lo - 2 * qp
                    nc.vector.tensor_add(
                        pscr[:, half * 512 + off:half * 512 + off + 128],
                        pscr[:, half * 512 + off:half * 512 + off + 128],
                        maskb[:, 0:128],
                    )
                lo = (512 if qc_lo == qcs[1] else 0) + \
                     (off if qc_lo in qcs else 0)
                expt = sb_exp.tile([128, 1024], FP16, tag="exp",
                                   name=f"ex{h}_{qp}_{t}")
                nc.scalar.activation(expt[:, lo:1024], pscr[:, lo:1024],
                                     AF.Exp)
                return expt

            def pv_step(h, qp, t, expt):
                qcs = (2 * qp, 2 * qp + 1)
                qc_lo = t // 4
                off = 128 * (t % 4)
                for half, qc in enumerate(qcs):
                    if qc < qc_lo:
                        continue
                    cs = off if qc == qc_lo else 0
                    nc.tensor.matmul(
                        pouts[(h, qc)][:, cs:512],
                        vp[:, h, t, :],
                        expt[:, half * 512 + cs:(half + 1) * 512],
                        start=(t == 0), stop=(t == 4 * qc + 3),
                    )

            def divide(h, qc):
                # evict the finished chain at once so its PSUM bank frees
                # immediately; the slow recip/divide runs off the copy
                pout = pouts.pop((h, qc))
                nout = sb_div.tile([DK + 1, 512], F32, tag="nout",
                                   name=f"no{h}_{qc}")
                nc.vector.tensor_copy(nout[:], pout[:])
                # spread the 512-wide den row over 64 partitions so the
                # expensive reciprocal runs 64 lanes wide, not 1
                rsp = sb_div.tile([DK, 8], F32, tag="rsp",
                                  name=f"rsp{h}_{qc}")
                nc.sync.dma_start(rsp[:], nout[DK:DK + 1, :])
                rcs = sb_div.tile([DK, 8], F32, tag="rcs",
                                  name=f"rcs{h}_{qc}")
                nc.vector.reciprocal(rcs[:], rsp[:])
                rc0 = sb_div.tile([1, 512], F32, tag="rc0",
                                  name=f"rc0{h}_{qc}")
                nc.sync.dma_start(rc0[:], rcs[:])
                rb = sb_div.tile([DK, 512], F32, tag="rb",
                                 name=f"rb{h}_{qc}")
                nc.gpsimd.partition_broadcast(rb[:], rc0[:])
                if h == 1:
                    # h1 lands at partitions 64-127: shift via SBUF DMA
                    tmp = sb_div.tile([DK, 512], FP16, tag="tmp",
                                      name=f"tmp{h}_{qc}")
                    nc.vector.tensor_mul(tmp[:], nout[0:DK, :], rb[:])
                    nc.sync.dma_start(
                        oct_[DK:128, 0, qc * 512:(qc + 1) * 512], tmp[:])
                else:
                    nc.vector.tensor_mul(
                        oct_[0:DK, h // 2, qc * 512:(qc + 1) * 512],
                        nout[0:DK, :], rb[:],
                    )

            # Flat pipelined stream over all (head, q-half, k-tile) steps:
            # scores run R steps ahead of PV so the PE always has queued
            # matmuls while ACT computes the exps (keeps the PE dense and
            # the HAM clock-gate warm). R=3 matches the 3-slot score pool.
            sc_list = [(h, qp, t) for h in range(HPC) for qp in range(2)
                       for t in range(8 * qp + 8)]
            R = 3
            expts = {}
            for k in range(len(sc_list) + R):
                if k < len(sc_list):
                    expts[sc_list[k]] = score_step(*sc_list[k])
                j = k - R
                if j < 0:
                    continue
                h, qp, t = sc_list[j]
                if t == 0:
                    for qc in (2 * qp, 2 * qp + 1):
                        pouts[(h, qc)] = ps_o.tile([DK + 1, 512], F32,
                                                   tag="pout",
                                                   name=f"po{h}_{qc}")
                pv_step(h, qp, t, expts.pop(sc_list[j]))
                if t == 4 * (2 * qp) + 3:
                    divide(h, 2 * qp)      # low chain done: free its bank
                if t == 4 * (2 * qp + 1) + 3:
                    divide(h, 2 * qp + 1)

            # ---- output projection, reusing the attention pools' slots
            for qt in range(NT):
                pp = ps_s.tile([128, D], F32, tag="scr", name=f"pp{qt}")
                for c, kk in ((0, 128), (1, DK)):
                    for js, je in ((0, 512), (512, D)):
                        nc.tensor.matmul(
                            pp[:, js:je],
                            oct_[0:kk, c, qt * 128:(qt + 1) * 128],
                            wot[0:kk, c, js:je],
                            start=(c == 0), stop=(c == 1),
                        )
                ot = sb_exp.tile([128, D], FP16, tag="exp", name=f"ot{qt}")
                nc.vector.tensor_copy(ot[:], pp[:])
                nc.sync.dma_start(out_d[qt * 128:(qt + 1) * 128, :], ot[:])

        if dbg:
            nc.sync.dma_start(qkv_dbg, qkvt[:].bitcast(F32))
            nc.sync.dma_start(vp_dbg, vp[:].bitcast(F32))
            nc.sync.dma_start(oct_dbg, oct_[:].bitcast(F32))

    nc.compile()
    _NC_CACHE[key] = nc
    return nc


def make_in_maps(X, Wq, Wk, Wv, Wo):
    X = np.ascontiguousarray(np.asarray(X, dtype=np.float32))
    Wq = np.asarray(Wq, dtype=np.float32)
    Wk = np.asarray(Wk, dtype=np.float32)
    Wv = np.asarray(Wv, dtype=np.float32)
    Wo = np.asarray(Wo, dtype=np.float32)

    # causal additive-bias tiles: keep q >= k; rows=k (p), cols=q (f),
    # diagonal offset delta = 128*di: keep iff f >= p + delta
    p = np.arange(128)[:, None]
    f = np.arange(512)[None, :]
    mask = np.where(f[:, :128] >= p, 0.0, MASK_NEG).astype(np.float32)
    ident = np.eye(128, dtype=np.float32)
    vones = np.ones((128, HPC * NT), dtype=np.float16)

    in_maps = []
    for c in range(NCORES):
        b, hg = c // 4, c % 4
        gh = [hg * HPC + l for l in range(HPC)]
        q = [Wq[g * DK:(g + 1) * DK, :] / 8.0 for g in gh]
        k = [Wk[g * DK:(g + 1) * DK, :] for g in gh]
        v = [Wv[g * DK:(g + 1) * DK, :] for g in gh]
        wcat_rows = np.vstack([
            q[0], q[1], k[0], k[1], q[2], v[0], k[2], v[1], v[2],
            np.zeros((DK, D), dtype=np.float32),
        ])                                            # (640, 768)
        wcat = np.ascontiguousarray(wcat_rows.T.reshape(NI, 128, NM * 128))
        w0, w1, w2 = (Wo[:, g * DK:(g + 1) * DK].T for g in gh)
        wot = np.ascontiguousarray(np.stack([
            np.vstack([w0, w1]),
            np.vstack([w2, np.zeros((DK, D), dtype=np.float32)]),
        ]).astype(np.float16))                                     # (2, 128, 768)
        xt = np.ascontiguousarray(X[b].T.reshape(NI, 128, S))
        in_maps.append({
            "xt": xt, "wcat": wcat, "wot": wot,
            "mask": mask, "ident": ident, "vones": vones,
        })
    return in_maps


def _run(in_maps, trace=False, trace_cores=None):
    nc = build_nc()
    return bass_utils.run_bass_kernel_spmd(
        nc, in_maps, core_ids=list(range(NCORES)),
        trace=trace, trace_cores=trace_cores,
    )


def kernel(X, Wq, Wk, Wv, Wo):
    in_maps = make_in_maps(X, Wq, Wk, Wv, Wo)
    res = _run(in_maps, trace=False)
    out = np.zeros((B, S, D), dtype=np.float32)
    for c in range(NCORES):
        out[c // 4] += res.results[c]["out"]
    return out


# revision 4
# speedup vs baseline: 1.3448x; 1.2211x over previous
"""Causal multi-head self-attention (B=2, S=2048, D=768, H=12) on 8 TRN2 NeuronCores.

Sharding: core c = (batch b=c//4, head-group hg=c%4 of 3 heads).
Each core computes Q/K/V for its 3 heads, causal attention, and the partial
output projection sum_h out_h @ Wo[:, h]^T -> (S, D) in fp16. Host sums the
4 head-group partials per batch (the unshard step).

On-core dataflow (transposed (feature, seq) layout, f32r matmuls), arranged
as one globally-woven instruction stream that keeps TensorE dense (the HAM
clock-gate re-throttles the PE to 1.2 GHz after any ~3.4us window with idle
gaps, which doubles every matmul):

  region 0: QKV^T chains for s-chunks 0,1 (psum[m,s] += WcatT[i,m].T @ XT[i,s]
     per 512-col chunk, 8-bank rotation) + V' transposes. DMA descriptors for
     X/weights are generated on sync+scalar+gpsimd in parallel, first chunk
     prioritized.
  region 1: attention q-half 0 for all 3 heads (24 pipelined steps), woven
     with the QKV chains + V' transposes of s-chunks 2,3 (the PE filler that
     keeps it busy while ACT exps).
  region 2: attention q-half 1 (48 steps), woven with output projection
     blocks for q-chunks 0,1 as their divides complete; projection of
     q-chunks 2,3 trails at the end.

Attention step pipeline (scores run R=2 steps ahead of PV):
  scoresT[k,q] = KT.T @ QT per 1024-wide q-half -> exp on ACT -> fp16 expt
  (NO pre-exp mask: the diagonal window is zeroed AFTER exp by a DVE
  multiply with a 0/1 fp16 mask, so ACT never waits on DVE) ->
  PV: pout[qc] += V'[t].T @ expT (65 rows: 64 data + denominator).
  Per-(h,qc) epilogue: recip(den) -> broadcast -> numerator * recip.

PSUM: scores (128,1024)=2 banks x2 + 2 pout banks + 2 filler banks = 8.
"""

import numpy as np
from contextlib import ExitStack

import concourse.bass as bass
import concourse.tile as tile
from concourse import bacc, mybir
from concourse import bass_utils

F32 = mybir.dt.float32
F32R = mybir.dt.float32r
BF16 = mybir.dt.bfloat16
FP16 = mybir.dt.float16
AF = mybir.ActivationFunctionType

B, S, D, H = 2, 2048, 768, 12
DK = 64
HPC = 3            # heads per core
NCORES = 8
NI = D // 128      # 6 input-feature chunks
NM = 5             # output m-chunks of 128 (640 rows incl. 64 pad)
NT = S // 128      # 16 k-tiles
NQC = S // 512     # 4 q-chunks

# per-local-head (base_partition, m_chunk) in the QKVT buffer
QPOS = [(0, 0), (64, 0), (0, 2)]
KPOS = [(0, 1), (64, 1), (0, 3)]
VPOS = [(64, 2), (64, 3), (0, 4)]

_NC_CACHE = {}


def build_nc(dbg=False):
    key = ("nc", dbg)
    if key in _NC_CACHE:
        return _NC_CACHE[key]
    nc = bacc.Bacc("TRN2", target_bir_lowering=False, debug=False,
                   num_devices=NCORES)

    xt_d = nc.dram_tensor("xt", [NI, 128, S], F32R, kind="ExternalInput").ap()
    wcat_d = nc.dram_tensor("wcat", [NI, 128, NM * 128], F32R, kind="ExternalInput").ap()
    wot_d = nc.dram_tensor("wot", [2, 128, D], FP16, kind="ExternalInput").ap()
    mask_d = nc.dram_tensor("mask", [128, 128], FP16, kind="ExternalInput").ap()
    id_d = nc.dram_tensor("ident", [128, 128], F32R, kind="ExternalInput").ap()
    out_d = nc.dram_tensor("out", [S, D], FP16, kind="ExternalOutput").ap()

    with tile.TileContext(nc) as tc, ExitStack() as ctx:
        const = ctx.enter_context(tc.tile_pool(name="const", bufs=1))

        # persistent SBUF buffers
        xt = const.tile([128, NI, S], F32R)             # X^T
        wcat = const.tile([128, NI, NM * 128], F32R)    # W^T (QKV packed)
        wot = const.tile([128, 2, D], FP16)             # Wo^T [h0;h1],[h2;pad]
        mask01 = const.tile([128, 128], FP16)           # 0/1 causal window mask
        ident = const.tile([128, 128], F32R)
        qkvt = const.tile([128, NM, S], F32R)           # Q^T/K^T/V^T packed
        vp = const.tile([128, HPC, NT, DK + 1], FP16)   # V' = [V | ones]
        oct_ = const.tile([128, 2, S], FP16)            # packed out^T [h0;h1],[h2]
        qk16 = const.tile([128, 4, S], FP16)            # fp16 Q/K for attention

        # DMA issue order = arrival priority. Spread descriptor generation
        # over three otherwise-idle engine queues; region 0's critical path
        # (wcat m-chunk-0 columns + the first s-chunk of X) goes first.
        ENGS = (nc.sync, nc.scalar, nc.gpsimd)
        for i in range(NI):
            ENGS[i % 3].dma_start(wcat[:, i, 0:128], wcat_d[i][:, 0:128])
        for i in range(NI):
            ENGS[i % 3].dma_start(xt[:, i, 0:512], xt_d[i][:, 0:512])
        for i in range(NI):
            ENGS[i % 3].dma_start(wcat[:, i, 128:NM * 128],
                                  wcat_d[i][:, 128:NM * 128])
        nc.sync.dma_start(ident[:], id_d)
        nc.gpsimd.dma_start(mask01[:], mask_d)
        nc.sync.dma_start(wot[:], wot_d.rearrange("c p f -> p c f"))
        nc.vector.memset(vp[:, :, :, DK:DK + 1], 1.0)   # denominator ones col
        for sc in range(1, NQC):
            for i in range(NI):
                ENGS[(sc * NI + i) % 3].dma_start(
                    xt[:, i, sc * 512:(sc + 1) * 512],
                    xt_d[i][:, sc * 512:(sc + 1) * 512])

        # ---- QKV^T projection chain + V' transpose emitters (shared by
        # region 0 and the weave)
        def qkv_chain(pool, tag, sc, m):
            s0 = sc * 512
            pq = pool.tile([128, 512], F32, tag=tag, name=f"pq{sc}_{m}")
            for i in range(NI):
                nc.tensor.matmul(
                    pq[:],
                    wcat[:, i, m * 128:(m + 1) * 128],
                    xt[:, i, s0:s0 + 512],
                    start=(i == 0), stop=(i == NI - 1),
                )
            nc.vector.tensor_copy(qkvt[:, m, s0:s0 + 512], pq[:])
            # fp16 shadow of Q/K rows for the attention core
            if m <= 1:
                nc.vector.tensor_copy(qk16[:, m, s0:s0 + 512], pq[:])
            elif m <= 3:
                nc.vector.tensor_copy(qk16[0:DK, m, s0:s0 + 512], pq[0:DK, :])

        def v_transposes(pool, tag, sc, h):
            vb, vchunk = VPOS[h]
            for t in range(4 * sc, 4 * sc + 4):
                ptr = pool.tile([128, DK], F32R, tag=tag, name=f"tr{h}_{t}")
                nc.tensor.transpose(
                    ptr[:],
                    qkvt[vb:vb + DK, vchunk, t * 128:(t + 1) * 128],
                    ident[vb:vb + DK, vb:vb + DK],
                )
                nc.vector.tensor_copy(vp[:, h, t, 0:DK], ptr[:])

        # ---- region 0: s-chunks 0,1 on a deep 8-bank rotation
        with tc.tile_pool(name="ps_ab", bufs=8, space="PSUM") as ps_ab:
            for sc in (0, 1):
                for m in range(NM):
                    qkv_chain(ps_ab, "ab", sc, m)
                for h in range(HPC):
                    v_transposes(ps_ab, "ab", sc, h)

        # ---- regions 1+2: woven attention + trailing QKV + projection
        with tc.tile_pool(name="ps_s", bufs=2, space="PSUM") as ps_s, \
             tc.tile_pool(name="ps_o", bufs=2, space="PSUM") as ps_o, \
             tc.tile_pool(name="ps_f", bufs=2, space="PSUM") as ps_f, \
             tc.tile_pool(name="sb_exp", bufs=6) as sb_exp, \
             tc.tile_pool(name="sb_div", bufs=3) as sb_div:

            pouts = {}

            def score_step(h, qp, t):
                qb, qchunk = QPOS[h]
                kb, kchunk = KPOS[h]
                qcs = (2 * qp, 2 * qp + 1)
                qc_lo = t // 4
                off = 128 * (t % 4)   # diag col offset inside qc_lo's half
                pscr = ps_s.tile([128, 1024], F32, tag="scr",
                                 name=f"sc{h}_{qp}_{t}")
                for half, qc in enumerate(qcs):
                    if qc < qc_lo:
                        continue
                    cs = off if qc == qc_lo else 0  # skip fully-masked cols
                    nc.tensor.matmul(
                        pscr[:, half * 512 + cs:(half + 1) * 512],
                        qk16[kb:kb + DK, kchunk, t * 128:(t + 1) * 128],
                        qk16[qb:qb + DK, qchunk,
                             qc * 512 + cs:(qc + 1) * 512],
                        start=True, stop=True,
                    )
                lo = (512 if qc_lo == qcs[1] else 0) + \
                     (off if qc_lo in qcs else 0)
                expt = sb_exp.tile([128, 1024], FP16, tag="exp",
                                   name=f"ex{h}_{qp}_{t}")
                nc.scalar.activation(expt[:, lo:1024], pscr[:, lo:1024],
                                     AF.Exp)
                if qc_lo in qcs:
                    # zero the above-diagonal weights AFTER exp (0/1 fp16
                    # mask on DVE) so ACT never waits on another engine
                    w = (qc_lo - 2 * qp) * 512 + off
                    nc.vector.tensor_mul(expt[:, w:w + 128],
                                         expt[:, w:w + 128], mask01[:])
                return expt

            def pv_step(h, qp, t, expt):
                qcs = (2 * qp, 2 * qp + 1)
                qc_lo = t // 4
                off = 128 * (t % 4)
                for half, qc in enumerate(qcs):
                    if qc < qc_lo:
                        continue
                    cs = off if qc == qc_lo else 0
                    nc.tensor.matmul(
                        pouts[(h, qc)][:, cs:512],
                        vp[:, h, t, :],
                        expt[:, half * 512 + cs:(half + 1) * 512],
                        start=(t == 0), stop=(t == 4 * qc + 3),
                    )

            def divide(h, qc):
                # evict the finished chain at once so its PSUM bank frees
                # immediately; the slow recip/divide runs off the copy
                pout = pouts.pop((h, qc))
                nout = sb_div.tile([DK + 1, 512], F32, tag="nout",
                                   name=f"no{h}_{qc}")
                nc.vector.tensor_copy(nout[:], pout[:])
                # spread the 512-wide den row over 64 partitions so the
                # expensive reciprocal runs 64 lanes wide, not 1
                rsp = sb_div.tile([DK, 8], F32, tag="rsp",
                                  name=f"rsp{h}_{qc}")
                nc.sync.dma_start(rsp[:], nout[DK:DK + 1, :])
                rcs = sb_div.tile([DK, 8], F32, tag="rcs",
                                  name=f"rcs{h}_{qc}")
                nc.vector.reciprocal(rcs[:], rsp[:])
                rc0 = sb_div.tile([1, 512], F32, tag="rc0",
                                  name=f"rc0{h}_{qc}")
                nc.sync.dma_start(rc0[:], rcs[:])
                rb = sb_div.tile([DK, 512], F32, tag="rb",
                                 name=f"rb{h}_{qc}")
                nc.gpsimd.partition_broadcast(rb[:], rc0[:])
                if h == 1:
                    # h1 lands at partitions 64-127: shift via SBUF DMA
                    tmp = sb_div.tile([DK, 512], FP16, tag="tmp",
                                      name=f"tmp{h}_{qc}")
                    nc.vector.tensor_mul(tmp[:], nout[0:DK, :], rb[:])
                    nc.sync.dma_start(
                        oct_[DK:128, 0, qc * 512:(qc + 1) * 512], tmp[:])
                else:
                    nc.vector.tensor_mul(
                        oct_[0:DK, h // 2, qc * 512:(qc + 1) * 512],
                        nout[0:DK, :], rb[:],
                    )

            def d_proj(qt):
                # output projection for one 128-row q-tile, split into two
                # single-bank psum tiles so it can borrow the filler pool
                pa = ps_f.tile([128, 512], F32, tag="fil", name=f"pa{qt}")
                pb = ps_f.tile([128, 256], F32, tag="fil", name=f"pb{qt}")
                for c, kk in ((0, 128), (1, DK)):
                    nc.tensor.matmul(pa[:], oct_[0:kk, c, qt * 128:(qt + 1) * 128],
                                     wot[0:kk, c, 0:512],
                                     start=(c == 0), stop=(c == 1))
                for c, kk in ((0, 128), (1, DK)):
                    nc.tensor.matmul(pb[:], oct_[0:kk, c, qt * 128:(qt + 1) * 128],
                                     wot[0:kk, c, 512:D],
                                     start=(c == 0), stop=(c == 1))
                ot = sb_exp.tile([128, D], FP16, tag="exp", name=f"ot{qt}")
                nc.vector.tensor_copy(ot[:, 0:512], pa[:])
                nc.vector.tensor_copy(ot[:, 512:D], pb[:])
                nc.sync.dma_start(out_d[qt * 128:(qt + 1) * 128, :], ot[:])

            # PE filler work, scheduled at fixed pipeline positions k:
            #   region 1 (k 0..23, attention q-half 0): QKV chains + V'
            #   transposes of s-chunks 2,3 (sc3's Q chunks m0/m2 must land
            #   before k=24 when q-half-1 scores start).
            #   region 2: projection of q-chunks 0,1 once all heads' divides
            #   for them completed (k>=26 / k>=40); q-chunk 2 near the end.
            fills = {}
            for n, m in enumerate(range(NM)):
                fills.setdefault(1 + 2 * n, []).append(
                    lambda m=m: qkv_chain(ps_f, "fil", 2, m))
            for h in range(HPC):
                fills.setdefault(11 + h, []).append(
                    lambda h=h: v_transposes(ps_f, "fil", 2, h))
            for n, m in enumerate(range(NM)):
                fills.setdefault(14 + 2 * n, []).append(
                    lambda m=m: qkv_chain(ps_f, "fil", 3, m))
            for h in range(HPC):
                fills.setdefault(25 + h, []).append(
                    lambda h=h: v_transposes(ps_f, "fil", 3, h))
            for n, qt in enumerate(range(0, 4)):        # q-chunk 0
                fills.setdefault(28 + 3 * n, []).append(
                    lambda qt=qt: d_proj(qt))
            for n, qt in enumerate(range(4, 8)):        # q-chunk 1
                fills.setdefault(40 + 3 * n, []).append(
                    lambda qt=qt: d_proj(qt))
            for n, qt in enumerate(range(8, 12)):       # q-chunk 2
                fills.setdefault(70 + n, []).append(
                    lambda qt=qt: d_proj(qt))

            # Flat pipelined stream over all (head, q-half, k-tile) steps,
            # q-half-major so region 1 only needs s-chunks 0,1. Scores run
            # R=2 ahead of PV (matching the 2-slot score pool) so the PE
            # always has queued matmuls while ACT computes the exps.
            sc_list = [(h, 0, t) for h in range(HPC) for t in range(8)] + \
                      [(h, 1, t) for h in range(HPC) for t in range(16)]
            R = 2
            expts = {}
            for k in range(len(sc_list) + R):
                if k < len(sc_list):
                    expts[sc_list[k]] = score_step(*sc_list[k])
                j = k - R
                if j >= 0:
                    h, qp, t = sc_list[j]
                    if t == 0:
                        for qc in (2 * qp, 2 * qp + 1):
                            pouts[(h, qc)] = ps_o.tile([DK + 1, 512], F32,
                                                       tag="pout",
                                                       name=f"po{h}_{qc}")
                    pv_step(h, qp, t, expts.pop(sc_list[j]))
                    if t == 4 * (2 * qp) + 3:
                        divide(h, 2 * qp)
                    if t == 4 * (2 * qp + 1) + 3:
                        divide(h, 2 * qp + 1)
                for fn in fills.pop(k, ()):
                    fn()

            for fns in sorted(fills):
                for fn in fills[fns]:
                    fn()
            for qt in range(12, 16):                    # q-chunk 3 trails
                d_proj(qt)

    nc.compile()
    _NC_CACHE[key] = nc
    return nc


def make_in_maps(X, Wq, Wk, Wv, Wo):
    X = np.ascontiguousarray(np.asarray(X, dtype=np.float32))
    Wq = np.asarray(Wq, dtype=np.float32)
    Wk = np.asarray(Wk, dtype=np.float32)
    Wv = np.asarray(Wv, dtype=np.float32)
    Wo = np.asarray(Wo, dtype=np.float32)

    # causal 0/1 window mask: keep q >= k; rows=k (p), cols=q (f)
    p = np.arange(128)[:, None]
    f = np.arange(128)[None, :]
    mask = (f >= p).astype(np.float16)
    ident = np.eye(128, dtype=np.float32)

    in_maps = []
    for c in range(NCORES):
        b, hg = c // 4, c % 4
        gh = [hg * HPC + l for l in range(HPC)]
        q = [Wq[g * DK:(g + 1) * DK, :] / 8.0 for g in gh]
        k = [Wk[g * DK:(g + 1) * DK, :] for g in gh]
        v = [Wv[g * DK:(g + 1) * DK, :] for g in gh]
        wcat_rows = np.vstack([
            q[0], q[1], k[0], k[1], q[2], v[0], k[2], v[1], v[2],
            np.zeros((DK, D), dtype=np.float32),
        ])                                            # (640, 768)
        wcat = np.ascontiguousarray(wcat_rows.T.reshape(NI, 128, NM * 128))
        w0, w1, w2 = (Wo[:, g * DK:(g + 1) * DK].T for g in gh)
        wot = np.ascontiguousarray(np.stack([
            np.vstack([w0, w1]),
            np.vstack([w2, np.zeros((DK, D), dtype=np.float32)]),
        ]).astype(np.float16))                                     # (2, 128, 768)
        xt = np.ascontiguousarray(X[b].T.reshape(NI, 128, S))
        in_maps.append({
            "xt": xt, "wcat": wcat, "wot": wot,
            "mask": mask, "ident": ident,
        })
    return in_maps


def _run(in_maps, trace=False, trace_cores=None):
    nc = build_nc()
    return bass_utils.run_bass_kernel_spmd(
        nc, in_maps, core_ids=list(range(NCORES)),
        trace=trace, trace_cores=trace_cores,
    )


def kernel(X, Wq, Wk, Wv, Wo):
    in_maps = make_in_maps(X, Wq, Wk, Wv, Wo)
    res = _run(in_maps, trace=False)
    out = np.zeros((B, S, D), dtype=np.float32)
    for c in range(NCORES):
        out[c // 4] += res.results[c]["out"]
    return out
